# revision 1
# baseline (speedup 1.0000x reference)
"""Trainium2 Bass kernel for nn_AttenMlpFinal (attention-MLP pooling).

Reference (per batch row b):
    xx[m]  = concat(q[b], k[b,m])                  # [2D]
    h      = relu(xx @ W1^T)                       # [M, H]
    scores = h @ W2^T                              # [M]
    attn   = softmax(scores over m)
    out[b] = sum_m attn[m] * v[b,m]                # [D]

Strategy (pure data parallel over bsz across 8 cores; all fp32):
  Host algebra:
    scores[b,m] = sum_h W2_h * relu(a_h),  a_h = q.W1q_h + k.W1k_h
    Split hidden units by sign of W2 (host permutation, neg group first):
      p_h  = |W2_h| * a_h          (positive-scaled pre-activation)
      scores = sum_pos relu(p) - sum_neg relu(p)
    NEG group (Q cols) -> ACT engine:  psum = a~ (= -p, sign-folded weights,
      q-term folded into psum by an extra PE matmul);
      relu(a~) summed by activation(Relu, accum_out).
      identity relu(-p) = relu(p) - p turns that into sum_neg relu(p) - sum_neg p.
    POS group (L cols) -> DVE engine: psum = pk (k-part only);
      relu(pk + pq) = max(pk, -pq) + pq, so one fused tensor_tensor_reduce
      (op0=max vs precomputed -pq, op1=add) with per-partition init carrying
      all linear corrections:
        init = q.wC + k.wbark,  wC = sum_h W2_h W1q_h,
        wbark = sum_{h: W2_h<0} W2_h W1k_h
      gives scores = TTR_accum - ACT_accum  (one tiny sub per 128-row block).
  softmax over m=8 without max subtraction (scores are O(1), fp32 exp exact).
  out = sum_m e_m v_m via PE: identity-stationary accumulating matmuls over
  v scaled by e_m (gpsimd tensor_scalar), then scale by 1/sum_m e on ACT.
  k and q are shipped pre-transposed (kT [D,M,B], qT [D,B]) so the
  contraction dim d sits on partitions with zero on-chip transposes.
  All matmuls use float32r (full fp32 data, 1 cycle/row for N>=256).
"""

import sys

sys.path.insert(0, "/opt/trn_rl_repo")

from contextlib import ExitStack

import numpy as np

import concourse.bass as bass
import concourse.tile as tile
from concourse import bacc, mybir
from concourse.bass_utils import run_bass_kernel_spmd

F32 = mybir.dt.float32
F32R = mybir.dt.float32r
ALU = mybir.AluOpType
ACTF = mybir.ActivationFunctionType

N_CORES = 8
BSZ, M, D, H = 32768, 8, 128, 512
B = BSZ // N_CORES  # rows per core

GROUP = 4  # b-blocks per v-sum matmul group (psum bank = 4*128 fp32 cols)


def r(ap):
    return ap.bitcast(F32R)


def build_nc(b_per_core: int, Q: int):
    """Q = size of the ACT-side (sign-folded) hidden group, must be >= 256
    so the float32r q-fold matmul streams at 1 cycle/row."""
    L = H - Q  # DVE-side group size
    nb = b_per_core // 128
    ngroups = nb // GROUP
    assert nb % GROUP == 0

    nc = bacc.Bacc("TRN2", target_bir_lowering=False, debug=False)

    kT = nc.declare_dram_parameter("kT", [D, M, b_per_core], F32R, isOutput=False)
    qT = nc.declare_dram_parameter("qT", [D, b_per_core], F32R, isOutput=False)
    v = nc.declare_dram_parameter("v", [b_per_core, M * D], F32, isOutput=False)
    # wk_rhs: W~k^T = (diag(W2) W1k)^T, hidden-permuted neg-first  [D, H]
    wk_rhs = nc.declare_dram_parameter("wk_rhs", [D, H], F32R, isOutput=False)
    # wq_all: W~q^T (same permutation) [D, H]; wq_neg = -wq_all
    wq_all = nc.declare_dram_parameter("wq_all", [D, H], F32R, isOutput=False)
    wq_neg = nc.declare_dram_parameter("wq_neg", [D, H], F32R, isOutput=False)
    wbark = nc.declare_dram_parameter("wbark", [D, 2], F32R, isOutput=False)
    wc8 = nc.declare_dram_parameter("wc8", [D, 2 * M], F32R, isOutput=False)
    ident = nc.declare_dram_parameter("ident", [128, 128], F32R, isOutput=False)
    out = nc.declare_dram_parameter("out", [b_per_core, D], F32, isOutput=True)

    with tile.TileContext(nc) as tc, ExitStack() as ctx:
        dram = ctx.enter_context(tc.tile_pool(name="dram", bufs=1, space="DRAM"))
        consts = ctx.enter_context(tc.tile_pool(name="consts", bufs=1))
        qpool = ctx.enter_context(tc.tile_pool(name="qpool", bufs=1))
        kpool = ctx.enter_context(tc.tile_pool(name="kpool", bufs=2))
        vpool = ctx.enter_context(tc.tile_pool(name="vpool", bufs=2))
        aqmp = ctx.enter_context(tc.tile_pool(name="aqmp", bufs=2))
        scr = ctx.enter_context(tc.tile_pool(name="scr", bufs=4))
        scra = ctx.enter_context(tc.tile_pool(name="scra", bufs=4))
        smax = ctx.enter_context(tc.tile_pool(name="smax", bufs=2 * GROUP + 2))
        vsc = ctx.enter_context(tc.tile_pool(name="vsc", bufs=2))
        outp = ctx.enter_context(tc.tile_pool(name="outp", bufs=2))

        ps_a = ctx.enter_context(tc.tile_pool(name="ps_a", bufs=2, space="PSUM"))
        ps_aqm = ctx.enter_context(tc.tile_pool(name="ps_aqm", bufs=1, space="PSUM"))
        ps_kc = ctx.enter_context(tc.tile_pool(name="ps_kc", bufs=2, space="PSUM"))
        ps_vo = ctx.enter_context(tc.tile_pool(name="ps_vo", bufs=2, space="PSUM"))

        # ---- constants / whole-q load ----
        wk_sb = consts.tile([D, H], F32R, tag="wk")
        nc.sync.dma_start(out=wk_sb[:], in_=wk_rhs[:])
        wqa_sb = consts.tile([D, H], F32R, tag="wqa")
        nc.sync.dma_start(out=wqa_sb[:], in_=wq_all[:])
        wqn_sb = consts.tile([D, H], F32R, tag="wqn")
        nc.sync.dma_start(out=wqn_sb[:], in_=wq_neg[:])
        wbk_sb = consts.tile([D, 2], F32R, tag="wbk")
        nc.sync.dma_start(out=wbk_sb[:], in_=wbark[:])
        wc8_sb = consts.tile([D, 2 * M], F32R, tag="wc8")
        nc.sync.dma_start(out=wc8_sb[:], in_=wc8[:])
        id_sb = consts.tile([128, 128], F32R, tag="ident")
        nc.sync.dma_start(out=id_sb[:], in_=ident[:])

        # Stage big inputs into internal DRAM: external (PJRT) buffers read
        # ~7x slower from SBUF-DMA than internal DRAM tensors; the bulk
        # DRAM->DRAM copy is fast.
        kT_i = dram.tile([D, M, b_per_core], F32R, name="kT_i")
        nc.sync.dma_start(out=kT_i[:], in_=kT[:])
        qT_i = dram.tile([D, b_per_core], F32R, name="qT_i")
        nc.sync.dma_start(out=qT_i[:], in_=qT[:])
        v_i = dram.tile([b_per_core, M * D], F32, name="v_i")
        nc.sync.dma_start(out=v_i[:], in_=v[:])

        qT_sb = qpool.tile([D, b_per_core], F32R)
        nc.sync.dma_start(out=qT_sb[:], in_=qT_i[:])

        for g in range(ngroups):
            gb = g * GROUP * 128  # first b row of this group

            kT_sb = kpool.tile([D, M, GROUP * 128], F32R)
            nc.sync.dma_start(out=kT_sb[:], in_=kT_i[:, :, gb : gb + GROUP * 128])
            v_sb = vpool.tile([128, GROUP, M * D], F32)
            for j in range(GROUP):
                nc.sync.dma_start(
                    out=v_sb[:, j, :], in_=v_i[gb + j * 128 : gb + (j + 1) * 128, :]
                )

            vscaled = [
                vsc.tile([128, GROUP, 128], F32R, tag=f"vs{m}", name=f"vs{m}")
                for m in range(M)
            ]
            recips = []

            for j in range(GROUP):
                qsl = qT_sb[:, gb + j * 128 : gb + (j + 1) * 128]

                # ---- q phase ----
                aqm_ps = ps_aqm.tile([128, H], F32)
                nc.tensor.matmul(
                    aqm_ps[:], qsl, wqn_sb[:], start=True, stop=True
                )
                aqm_sb = aqmp.tile([128, L], F32)
                nc.scalar.copy(aqm_sb[:], aqm_ps[:, Q:H])
                klc = ps_kc.tile([128, M, 2], F32)
                nc.tensor.matmul(
                    klc[:, :, :], qsl, wc8_sb[:], start=True, stop=False,
                    skip_group_check=True,
                )
                # k-linear corrections for all m
                for m in range(M):
                    ksl = kT_sb[:, m, j * 128 : (j + 1) * 128]
                    nc.tensor.matmul(
                        klc[:, m, :], ksl, wbk_sb[:],
                        start=False, stop=(m == M - 1), skip_group_check=True,
                    )

                sc_d = smax.tile([128, M], F32, tag="sc_d")
                sc_a = smax.tile([128, M], F32, tag="sc_a")

                # ---- per-m main work ----
                for m in range(M):
                    ksl = kT_sb[:, m, j * 128 : (j + 1) * 128]
                    a_ps = ps_a.tile([128, H], F32)
                    nc.tensor.matmul(
                        a_ps[:], ksl, wk_sb[:], start=True, stop=False
                    )
                    # fold q into the ACT-side (neg-group) columns
                    nc.tensor.matmul(
                        a_ps[:], qsl, wqa_sb[:],
                        start=False, stop=True,
                    )
                    t_a = scra.tile([128, Q], F32, tag="scra")
                    nc.scalar.activation(
                        t_a[:], a_ps[:, 0:Q], ACTF.Relu,
                        accum_out=sc_a[:, m : m + 1],
                    )
                    t_d = scr.tile([128, L], F32, tag="scr")
                    nc.vector.scalar_tensor_tensor(
                        out=t_d[:],
                        in0=a_ps[:, Q:H],
                        scalar=0.0,
                        in1=aqm_sb[:],
                        op0=ALU.bypass,
                        op1=ALU.max,
                        accum_out=sc_d[:, m : m + 1],
                    )

                # ---- scores = sc_d - sc_a ; softmax over m ----
                scores = smax.tile([128, M], F32, tag="scores")
                nc.vector.tensor_sub(scores[:], sc_d[:], sc_a[:])
                nc.vector.tensor_add(scores[:], scores[:], klc[:, :, 0])
                e_sb = smax.tile([128, M], F32, tag="e")
                nc.scalar.activation(e_sb[:], scores[:], ACTF.Exp)
                denom = smax.tile([128, 1], F32, tag="denom")
                nc.vector.tensor_reduce(
                    denom[:], e_sb[:], mybir.AxisListType.X, ALU.add
                )
                rec = smax.tile([128, 1], F32, tag="recip")
                nc.vector.reciprocal(rec[:], denom[:])
                recips.append(rec)

                # ---- scale v by e_m (gpsimd; v and e in SBUF) ----
                for m in range(M):
                    nc.vector.tensor_scalar_mul(
                        vscaled[m][:, j, :],
                        v_sb[:, j, m * 128 : (m + 1) * 128],
                        e_sb[:, m : m + 1],
                    )

            # ---- v-sum via identity-stationary accumulating matmuls ----
            vo_ps = ps_vo.tile([128, GROUP * 128], F32)
            for m in range(M):
                nc.tensor.matmul(
                    vo_ps[:],
                    id_sb[:],
                    vscaled[m][:, :, :],
                    start=(m == 0),
                    stop=(m == M - 1),
                )

            out_sb = outp.tile([128, GROUP, 128], F32)
            for j in range(GROUP):
                nc.scalar.mul(
                    out_sb[:, j, :],
                    vo_ps[:, j * 128 : (j + 1) * 128],
                    recips[j][:],
                )
            for j in range(GROUP):
                nc.sync.dma_start(
                    out=out[gb + j * 128 : gb + (j + 1) * 128, :],
                    in_=out_sb[:, j, :],
                )

    nc.compile()
    return nc


def host_prep(q_vec, k_vec, v_vec, W1, W2, b_per_core):
    """Host-side resharding + weight preprocessing (numpy only)."""
    W1 = np.asarray(W1, dtype=np.float32)
    W2 = np.asarray(W2, dtype=np.float32).reshape(-1)  # [H]

    neg = W2 < 0
    # ACT group = majority sign group; must have >= 256 columns for f32r.
    act_is_neg = neg.sum() >= (H // 2)
    grp_a = neg if act_is_neg else ~neg
    order = np.concatenate([np.where(grp_a)[0], np.where(~grp_a)[0]])
    Q = int(grp_a.sum())
    assert Q >= 256, f"ACT group too small: {Q}"

    Wt = (W1 * W2[:, None])[order]  # diag(W2) @ W1, permuted  [H, 2D]
    Wabs = (W1 * np.abs(W2)[:, None])[order]  # |W2| scaled, permuted
    # k-side rhs: ACT cols signed (Wt), DVE cols positive (Wabs)
    wk = np.concatenate([Wt[:Q, D:], Wabs[Q:, D:]], axis=0)  # [H, D]
    wq = np.concatenate([Wt[:Q, :D], Wabs[Q:, :D]], axis=0)  # [H, D]

    if act_is_neg:
        # scores = TTR(init) - ACT_accum
        # init = q.wC + k.wbark; wC = sum_h W2_h W1q_h; wbark = sum_neg W2_h W1k_h
        wC = (W1[:, :D] * W2[:, None]).sum(axis=0)
        wbk = (W1[neg, D:] * W2[neg, None]).sum(axis=0)
        sub_act = True
    else:
        # ACT group positive: scores = ACT_accum + TTR(init); init = -sum_neg pq
        wC = -(W1[neg, :D] * np.abs(W2[neg, None])).sum(axis=0)
        wbk = np.zeros(D, dtype=np.float32)
        sub_act = False

    wk_rhs = np.ascontiguousarray(wk.T, dtype=np.float32)  # [D, H]
    wq_pad = wq.T.copy()
    wq_pad[:, Q:] = 0.0  # q-fold touches only the ACT-side columns
    wq_all = np.ascontiguousarray(wq_pad, dtype=np.float32)  # [D, H]
    wq_neg = np.ascontiguousarray(-wq.T, dtype=np.float32)  # [D, H]
    wbark = np.zeros((D, 2), dtype=np.float32)
    wbark[:, 0] = wbk
    wc8 = np.zeros((D, 2 * M), dtype=np.float32)
    wc8[:, 0::2] = wC.astype(np.float32)[:, None]
    ident = np.eye(128, dtype=np.float32)

    in_maps = []
    n_cores = len(q_vec) // b_per_core
    for c in range(n_cores):
        sl = slice(c * b_per_core, (c + 1) * b_per_core)
        k_sh = np.asarray(k_vec[sl], dtype=np.float32)
        q_sh = np.asarray(q_vec[sl], dtype=np.float32)
        v_sh = np.asarray(v_vec[sl], dtype=np.float32)
        in_maps.append(
            {
                "kT": np.ascontiguousarray(k_sh.transpose(2, 1, 0)),  # [D, M, B]
                "qT": np.ascontiguousarray(q_sh.T),  # [D, B]
                "v": np.ascontiguousarray(v_sh.reshape(b_per_core, M * D)),
                "wk_rhs": wk_rhs,
                "wq_all": wq_all,
                "wq_neg": wq_neg,
                "wbark": wbark,
                "wc8": wc8,
                "ident": ident,
            }
        )
    return in_maps, Q, sub_act


_NC_CACHE = {}


def kernel(q_vec, k_vec, v_vec, W1, W2):
    in_maps, Q, sub_act = host_prep(q_vec, k_vec, v_vec, W1, W2, B)
    assert sub_act, "kernel built for neg-majority W2 (scores = TTR - ACT)"
    key = (B, Q)
    if key not in _NC_CACHE:
        _NC_CACHE[key] = build_nc(B, Q)
    nc = _NC_CACHE[key]
    res = run_bass_kernel_spmd(nc, in_maps, list(range(N_CORES)))
    outs = [res.results[c]["out"] for c in range(N_CORES)]
    return np.ascontiguousarray(np.concatenate(outs, axis=0), dtype=np.float32)


if __name__ == "__main__":
    rng = np.random.default_rng(0)
    q = rng.standard_normal((BSZ, D), dtype=np.float32)
    k = rng.standard_normal((BSZ, M, D), dtype=np.float32)
    v = rng.standard_normal((BSZ, M, D), dtype=np.float32)
    W1 = (rng.standard_normal((H, 2 * D)) / np.sqrt(2 * D)).astype(np.float32)
    W2 = (rng.standard_normal((1, H)) / np.sqrt(H)).astype(np.float32)
    o = kernel(q, k, v, W1, W2)
    print(o.shape, o.dtype)



# revision 2
# speedup vs baseline: 1.5851x; 1.5851x over previous
"""Trainium2 Bass kernel for nn_AttenMlpFinal (attention-MLP pooling).

Reference (per batch row b):
    xx[m]  = concat(q[b], k[b,m])                  # [2D]
    h      = relu(xx @ W1^T)                       # [M, H]
    scores = h @ W2^T                              # [M]
    attn   = softmax(scores over m)
    out[b] = sum_m attn[m] * v[b,m]                # [D]

Strategy (pure data parallel over bsz across 8 cores; bf16 matmul inputs):
  Fold |W2_h| into W1 row h (relu scale-invariance), permute hidden units
  into three groups [act(neg) | min(neg) | max(pos)]:
    scores[b,m] = sum_pos max(K_h, -P_h) + sum_negDVE min(-K_h, P_h)
                  - sum_negACT relu(P_h + K_h)   (+ const(b) dropped:
                  q-only linear terms are constant over m and cancel in
                  softmax, so no q-replay matmuls or linear corrections).
  where P = q-side preactivation, K = k-side preactivation (|W2|-scaled).
  Engines:
    PE  (bf16, FWL): K = k.WK per (block,m); q-fold only for the ACT
        group's Q cols; nPQ = q.WQn once per block; v-sum via
        identity-stationary accumulating matmuls over attn-scaled v.
    ACT: relu+accum on the act group (Q cols, full preact in PSUM);
         psum->sbuf copies; exp.
    DVE: scalar_tensor_tensor min/max with accum on the other L cols
         (in0 = PSUM K, in1 = nPQ in SBUF); softmax combine; attn-scale
         of v in bf16 (4x packed mode).
  softmax over m=8 without max subtraction (scores are O(1)).
  k and q ship pre-transposed (kT [D,M,B], qT [D,B]) so the contraction
  dim d sits on partitions with zero on-chip transposes.
"""

import sys

sys.path.insert(0, "/opt/trn_rl_repo")

from contextlib import ExitStack

import numpy as np
import ml_dtypes

import concourse.bass as bass
import concourse.tile as tile
from concourse import bacc, mybir
from concourse.bass_utils import run_bass_kernel_spmd

F32 = mybir.dt.float32
BF16 = mybir.dt.bfloat16
ALU = mybir.AluOpType
ACTF = mybir.ActivationFunctionType

N_CORES = 8
BSZ, M, D, H = 32768, 8, 128, 512
B = BSZ // N_CORES  # rows per core

GROUP = 4  # b-blocks per v-sum matmul group (psum bank = 4*128 fp32 cols)
Q_TARGET = 208  # ACT-side hidden group size (tunable; <= #neg(W2))

BF = ml_dtypes.bfloat16


def build_nc(b_per_core: int, Q: int, r: int):
    """Q = ACT group size, r = DVE min-group size; L = H - Q total DVE cols."""
    L = H - Q
    Hp = L - r  # DVE max-group size
    nb = b_per_core // 128
    ngroups = nb // GROUP
    assert nb % GROUP == 0

    nc = bacc.Bacc("TRN2", target_bir_lowering=False, debug=False)

    kT = nc.declare_dram_parameter("kT", [D, M, b_per_core], BF16, isOutput=False)
    qT = nc.declare_dram_parameter("qT", [D, b_per_core], BF16, isOutput=False)
    v = nc.declare_dram_parameter("v", [b_per_core, M * D], BF16, isOutput=False)
    wk = nc.declare_dram_parameter("wk", [D, H], BF16, isOutput=False)
    wqa = nc.declare_dram_parameter("wqa", [D, Q], BF16, isOutput=False)
    wqn = nc.declare_dram_parameter("wqn", [D, L], BF16, isOutput=False)
    ident = nc.declare_dram_parameter("ident", [128, 128], BF16, isOutput=False)
    out = nc.declare_dram_parameter("out", [b_per_core, D], F32, isOutput=True)

    with tile.TileContext(nc) as tc, ExitStack() as ctx:
        dram = ctx.enter_context(tc.tile_pool(name="dram", bufs=1, space="DRAM"))
        consts = ctx.enter_context(tc.tile_pool(name="consts", bufs=1))
        qpool = ctx.enter_context(tc.tile_pool(name="qpool", bufs=1))
        kpool = ctx.enter_context(tc.tile_pool(name="kpool", bufs=2))
        vpool = ctx.enter_context(tc.tile_pool(name="vpool", bufs=2))
        npqp = ctx.enter_context(tc.tile_pool(name="npqp", bufs=2))
        scr = ctx.enter_context(tc.tile_pool(name="scr", bufs=4))
        smax = ctx.enter_context(tc.tile_pool(name="smax", bufs=2 * GROUP + 2))
        vsc = ctx.enter_context(tc.tile_pool(name="vsc", bufs=2))
        outp = ctx.enter_context(tc.tile_pool(name="outp", bufs=2))

        ps_a = ctx.enter_context(tc.tile_pool(name="ps_a", bufs=3, space="PSUM"))
        ps_npq = ctx.enter_context(tc.tile_pool(name="ps_npq", bufs=2, space="PSUM"))
        ps_vo = ctx.enter_context(tc.tile_pool(name="ps_vo", bufs=2, space="PSUM"))

        # ---- constants ----
        wk_sb = consts.tile([D, H], BF16, tag="wk")
        nc.sync.dma_start(out=wk_sb[:], in_=wk[:])
        wqa_sb = consts.tile([D, Q], BF16, tag="wqa")
        nc.sync.dma_start(out=wqa_sb[:], in_=wqa[:])
        wqn_sb = consts.tile([D, L], BF16, tag="wqn")
        nc.sync.dma_start(out=wqn_sb[:], in_=wqn[:])
        id_sb = consts.tile([128, 128], BF16, tag="ident")
        nc.sync.dma_start(out=id_sb[:], in_=ident[:])

        # Stage big inputs into internal DRAM: external (PJRT) buffers read
        # ~7x slower from SBUF-DMA than internal DRAM tensors; the bulk
        # DRAM->DRAM copy is fast.
        kT_i = dram.tile([D, M, b_per_core], BF16, name="kT_i")
        nc.sync.dma_start(out=kT_i[:], in_=kT[:])
        qT_i = dram.tile([D, b_per_core], BF16, name="qT_i")
        nc.sync.dma_start(out=qT_i[:], in_=qT[:])
        v_i = dram.tile([b_per_core, M * D], BF16, name="v_i")
        nc.sync.dma_start(out=v_i[:], in_=v[:])

        qT_sb = qpool.tile([D, b_per_core], BF16)
        nc.sync.dma_start(out=qT_sb[:], in_=qT_i[:])

        for g in range(ngroups):
            gb = g * GROUP * 128  # first b row of this group

            kT_sb = kpool.tile([D, M, GROUP * 128], BF16)
            nc.sync.dma_start(out=kT_sb[:], in_=kT_i[:, :, gb : gb + GROUP * 128])
            v_sb = vpool.tile([128, GROUP, M * D], BF16)
            for j in range(GROUP):
                nc.sync.dma_start(
                    out=v_sb[:, j, :], in_=v_i[gb + j * 128 : gb + (j + 1) * 128, :]
                )

            vscaled = [
                vsc.tile([128, GROUP, 128], BF16, tag=f"vs{m}", name=f"vs{m}")
                for m in range(M)
            ]

            for j in range(GROUP):
                qsl = qT_sb[:, gb + j * 128 : gb + (j + 1) * 128]

                # ---- q phase: nPQ for the DVE groups ----
                npq_ps = ps_npq.tile([128, L], F32)
                nc.tensor.matmul(npq_ps[:], qsl, wqn_sb[:], start=True, stop=True)
                npq_sb = npqp.tile([128, L], BF16)
                nc.scalar.copy(npq_sb[:], npq_ps[:])

                sc_a = smax.tile([128, M], F32, tag="sc_a")
                sc_n = smax.tile([128, M], F32, tag="sc_n")
                sc_x = smax.tile([128, M], F32, tag="sc_x")

                # ---- per-m main work ----
                for m in range(M):
                    ksl = kT_sb[:, m, j * 128 : (j + 1) * 128]
                    a_ps = ps_a.tile([128, H], F32)
                    # ACT group: full preactivation P+K in cols :Q
                    nc.tensor.matmul(
                        a_ps[:, 0:Q], qsl, wqa_sb[:], start=True, stop=False
                    )
                    nc.tensor.matmul(
                        a_ps[:, 0:Q], ksl, wk_sb[:, 0:Q],
                        start=False, stop=True, skip_group_check=True,
                    )
                    # DVE groups: K only in cols Q:
                    nc.tensor.matmul(
                        a_ps[:, Q:H], ksl, wk_sb[:, Q:H],
                        start=True, stop=True, skip_group_check=True,
                    )
                    t_a = scr.tile([128, Q], BF16, tag="scra")
                    nc.scalar.activation(
                        t_a[:], a_ps[:, 0:Q], ACTF.Relu,
                        accum_out=sc_a[:, m : m + 1],
                    )
                    t_n = scr.tile([128, r], BF16, tag="scrn")
                    nc.vector.scalar_tensor_tensor(
                        out=t_n[:],
                        in0=a_ps[:, Q : Q + r],
                        scalar=0.0,
                        in1=npq_sb[:, 0:r],
                        op0=ALU.bypass,
                        op1=ALU.min,
                        accum_out=sc_n[:, m : m + 1],
                    )
                    t_x = scr.tile([128, Hp], BF16, tag="scrx")
                    nc.vector.scalar_tensor_tensor(
                        out=t_x[:],
                        in0=a_ps[:, Q + r : H],
                        scalar=0.0,
                        in1=npq_sb[:, r:L],
                        op0=ALU.bypass,
                        op1=ALU.max,
                        accum_out=sc_x[:, m : m + 1],
                    )

                # ---- scores = sc_x + sc_n - sc_a ; softmax over m ----
                scores = smax.tile([128, M], F32, tag="scores")
                nc.vector.tensor_sub(scores[:], sc_x[:], sc_a[:])
                nc.vector.tensor_add(scores[:], scores[:], sc_n[:])
                e_sb = smax.tile([128, M], F32, tag="e")
                nc.scalar.activation(e_sb[:], scores[:], ACTF.Exp)
                denom = smax.tile([128, 1], F32, tag="denom")
                nc.vector.tensor_reduce(
                    denom[:], e_sb[:], mybir.AxisListType.X, ALU.add
                )
                rec = smax.tile([128, 1], F32, tag="recip")
                nc.vector.reciprocal(rec[:], denom[:])
                attn = smax.tile([128, M], F32, tag="attn")
                nc.vector.tensor_scalar_mul(attn[:], e_sb[:], rec[:])

                # ---- scale v by attn_m (bf16, DVE packed mode) ----
                for m in range(M):
                    nc.vector.tensor_scalar_mul(
                        vscaled[m][:, j, :],
                        v_sb[:, j, m * 128 : (m + 1) * 128],
                        attn[:, m : m + 1],
                    )

            # ---- v-sum via identity-stationary accumulating matmuls ----
            vo_ps = ps_vo.tile([128, GROUP * 128], F32)
            for m in range(M):
                nc.tensor.matmul(
                    vo_ps[:],
                    id_sb[:],
                    vscaled[m][:, :, :],
                    start=(m == 0),
                    stop=(m == M - 1),
                )

            out_sb = outp.tile([128, GROUP, 128], F32)
            nc.scalar.copy(out_sb[:, :, :], vo_ps[:])
            for j in range(GROUP):
                nc.sync.dma_start(
                    out=out[gb + j * 128 : gb + (j + 1) * 128, :],
                    in_=out_sb[:, j, :],
                )

    nc.compile()
    return nc


def host_prep(q_vec, k_vec, v_vec, W1, W2, b_per_core):
    """Host-side resharding + weight preprocessing (numpy only)."""
    W1 = np.asarray(W1, dtype=np.float32)
    w2 = np.asarray(W2, dtype=np.float32).reshape(-1)  # [H]

    neg = w2 < 0
    neg_idx = np.where(neg)[0]
    pos_idx = np.where(~neg)[0]
    Q = min(Q_TARGET, len(neg_idx))
    act_idx = neg_idx[:Q]
    min_idx = neg_idx[Q:]
    r = len(min_idx)
    order = np.concatenate([act_idx, min_idx, pos_idx])

    Ws = (np.abs(w2)[:, None] * W1)[order]  # [H, 2D] |W2|-folded, permuted
    Wsq, Wsk = Ws[:, :D], Ws[:, D:]

    WK = np.ascontiguousarray(Wsk.T, dtype=np.float32)  # [D, H]
    WK[:, Q : Q + r] *= -1.0  # min-group psum holds -K
    WQa = np.ascontiguousarray(Wsq[:Q].T, dtype=np.float32)  # [D, Q]
    WQn = np.ascontiguousarray(Wsq[Q:].T, dtype=np.float32)  # [D, L]
    WQn[:, r:] *= -1.0  # max-group in1 = -P (min-group keeps +P)

    wk_b = WK.astype(BF)
    wqa_b = WQa.astype(BF)
    wqn_b = WQn.astype(BF)
    ident = np.eye(128, dtype=np.float32).astype(BF)

    in_maps = []
    n_cores = len(q_vec) // b_per_core
    for c in range(n_cores):
        sl = slice(c * b_per_core, (c + 1) * b_per_core)
        k_sh = np.asarray(k_vec[sl], dtype=np.float32)
        q_sh = np.asarray(q_vec[sl], dtype=np.float32)
        v_sh = np.asarray(v_vec[sl], dtype=np.float32)
        in_maps.append(
            {
                "kT": np.ascontiguousarray(k_sh.transpose(2, 1, 0)).astype(BF),
                "qT": np.ascontiguousarray(q_sh.T).astype(BF),
                "v": np.ascontiguousarray(v_sh.reshape(b_per_core, M * D)).astype(BF),
                "wk": wk_b,
                "wqa": wqa_b,
                "wqn": wqn_b,
                "ident": ident,
            }
        )
    return in_maps, Q, r


_NC_CACHE = {}


def kernel(q_vec, k_vec, v_vec, W1, W2):
    in_maps, Q, r = host_prep(q_vec, k_vec, v_vec, W1, W2, B)
    key = (B, Q, r)
    if key not in _NC_CACHE:
        _NC_CACHE[key] = build_nc(B, Q, r)
    nc = _NC_CACHE[key]
    res = run_bass_kernel_spmd(nc, in_maps, list(range(N_CORES)))
    outs = [res.results[c]["out"] for c in range(N_CORES)]
    return np.ascontiguousarray(np.concatenate(outs, axis=0), dtype=np.float32)


if __name__ == "__main__":
    rng = np.random.default_rng(0)
    q = rng.standard_normal((BSZ, D), dtype=np.float32)
    k = rng.standard_normal((BSZ, M, D), dtype=np.float32)
    v = rng.standard_normal((BSZ, M, D), dtype=np.float32)
    W1 = (rng.standard_normal((H, 2 * D)) / np.sqrt(2 * D)).astype(np.float32)
    W2 = (rng.standard_normal((1, H)) / np.sqrt(H)).astype(np.float32)
    o = kernel(q, k, v, W1, W2)
    print(o.shape, o.dtype)
